# revision 1
# baseline (speedup 1.0000x reference)
"""Trainium2 Bass kernel for DCNv2 modulated deformable conv + BN + ReLU.

Problem: x[4,128,128,128], 3x3 deformable conv (offsets/mask from a dense
3x3 conv), 1 deformable group, BN (inference) + ReLU.

Sharding: 8 cores = (batch b = core//2) x (row-half h = core%2).
Each core computes output rows [64h, 64h+64) of batch b.

Per-core pipeline:
  S1  transpose halo slab -> XT [x, y, c] bf16 in SBUF
  S2  write pair image XPD[y, x] = (pix(y,x), pix(y,x+1)) channels -> DRAM
  S3  offset conv (fp32r matmuls) -> om [27, pos]; PE-transpose -> OMT [pos, 27]
  S4  offset math on DVE -> slot-weight coefs (bf16) + gather indices (int16)
  S5  dma_gather pair rows from XPD (HBM, sample-major [pos, (2pix,c)])
  S6  bilinear combine: V = sum_q wq * plane_q   (DVE, bf16)
  S7  PE-transpose V -> [c, pos]; main matmul over (c,k) bf16 -> psum
  S8  ACT epilogue relu(psum*A + B) -> out rows -> DRAM
"""
import os
import numpy as np
import ml_dtypes
from contextlib import ExitStack

import concourse.bass as bass
import concourse.mybir as mybir
import concourse.tile as tile
from concourse import bacc
from concourse.masks import make_identity
from concourse import library_config

F32 = mybir.dt.float32
F32R = mybir.dt.float32r
BF16 = mybir.dt.bfloat16
I16 = mybir.dt.int16
I32 = mybir.dt.int32
AL = mybir.AluOpType
ACT = mybir.ActivationFunctionType

B, C, H, W = 4, 128, 128, 128
CO = 128
K2 = 9
HL = 88           # halo slab rows per core
RT = 64           # output rows per core
RB = 2            # rows per gather block
NBLK = RT // RB   # 32 blocks
GRP = RB * K2     # gather groups per block (18)
NK = RT * K2      # 576
EPS = 1e-5

_CACHE = {}


def _build_nc():
    nc = bacc.Bacc("TRN2", target_bir_lowering=False)

    # ---------------- I/O ----------------
    xp_d = nc.dram_tensor("xp", [C, 66 * 130], F32, kind="ExternalInput")
    xh_d = nc.dram_tensor("xh", [C, HL * W], F32, kind="ExternalInput")
    wom_d = nc.dram_tensor("wom", [C, K2 * 27], F32, kind="ExternalInput")
    bom_d = nc.dram_tensor("bom", [27, 1], F32, kind="ExternalInput")
    wl_d = nc.dram_tensor("wl", [C, K2 * CO], BF16, kind="ExternalInput")
    av_d = nc.dram_tensor("av", [CO, 1], F32, kind="ExternalInput")
    bv_d = nc.dram_tensor("bv", [CO, 1], F32, kind="ExternalInput")
    rk_d = nc.dram_tensor("rk", [128, NK], F32, kind="ExternalInput")   # 64h+r+ky-1
    kxx_d = nc.dram_tensor("kxx", [128, NK], F32, kind="ExternalInput")  # p+kx-1
    ybase_d = nc.dram_tensor("ybase", [128, 1], F32, kind="ExternalInput")

    yl_d = nc.dram_tensor("yl", [CO, RT * W], F32, kind="ExternalOutput")
    xpd_d = nc.dram_tensor("xpd", [HL * W, 256], BF16, kind="Internal")

    with ExitStack() as ctx:
        tc = ctx.enter_context(tile.TileContext(nc))
        cp = ctx.enter_context(tc.tile_pool(name="const", bufs=1))

        ep = ctx.enter_context(tc.tile_pool(name="early", bufs=1))
        # persistent tiles
        xt = ep.tile([128, HL * C], BF16)           # XT[x, y*128+c]
        omt = cp.tile([128, RT * 27], F32)          # OMT[p, r*27+ch]
        w00 = cp.tile([128, NK], BF16)
        w01 = cp.tile([128, NK], BF16)
        w10 = cp.tile([128, NK], BF16)
        w11 = cp.tile([128, NK], BF16)
        wr0 = cp.tile([128, NK * 8], I16)           # wrapped idx (y0 rows)
        wr1 = cp.tile([128, NK * 8], I16)           # wrapped idx (y1 rows)
        w_sb = cp.tile([128, K2 * CO], BF16)
        wom_sb = cp.tile([128, K2 * 27], F32)
        bom_sb = cp.tile([27, 1], F32)
        av_sb = cp.tile([CO, 1], F32)
        bv_sb = cp.tile([CO, 1], F32)
        rk_sb = cp.tile([128, NK], F32)
        kxx_sb = cp.tile([128, NK], F32)
        ybase_sb = cp.tile([128, 1], F32)
        idf = cp.tile([128, 128], F32)
        idb = cp.tile([128, 128], BF16)
        xp_sb = ep.tile([128, 66 * 130], F32)

        nc.gpsimd.load_library(library_config.mlp)
        nc.sync.dma_start(w_sb[:], wl_d[:])
        nc.sync.dma_start(wom_sb[:], wom_d[:])
        nc.sync.dma_start(bom_sb[:], bom_d[:])
        nc.sync.dma_start(av_sb[:], av_d[:])
        nc.sync.dma_start(bv_sb[:], bv_d[:])
        nc.sync.dma_start(rk_sb[:], rk_d[:])
        nc.sync.dma_start(kxx_sb[:], kxx_d[:])
        nc.sync.dma_start(ybase_sb[:], ybase_d[:])
        nc.sync.dma_start(xp_sb[:], xp_d[:])
        make_identity(nc, idf[:])
        make_identity(nc, idb[:])

        # ---------- S1: build XT (transpose halo slab, cast bf16) ----------
        with tc.tile_pool(name="s1", bufs=2) as s1, \
             tc.tile_pool(name="s1p", bufs=2, space="PSUM") as s1p:
            CH = 8
            for cidx in range(HL // CH):
                xh_sb = s1.tile([128, CH * W], F32, tag="xh")
                nc.sync.dma_start(xh_sb[:], xh_d[:, cidx * CH * W:(cidx + 1) * CH * W])
                for half in range(CH // 4):
                    pt = s1p.tile([128, 512], F32, tag="ptx")
                    for j in range(4):
                        row = half * 4 + j
                        nc.tensor.transpose(pt[:, j * 128:(j + 1) * 128],
                                            xh_sb[:, row * W:(row + 1) * W], idf[:])
                    y0r = cidx * CH + half * 4
                    nc.scalar.copy(xt[:, y0r * C:(y0r + 4) * C], pt[:])

        # ---------- S2: write XPD pair image to DRAM ----------
        xpd_4d = xpd_d.ap().rearrange("(y x) (s c) -> y x s c", x=128, s=2)
        xt_v = xt[:].rearrange("x (y c) -> x y c", y=HL)
        nc.sync.dma_start(xpd_4d[:, :, 0, :].rearrange("y x c -> x y c"), xt_v)
        nc.sync.dma_start(xpd_4d[:, 0:127, 1, :].rearrange("y x c -> x y c"),
                          xt_v[1:128])

        # ---------- S3: offset conv + OMT ----------
        xp_v = xp_sb[:].rearrange("c (r x) -> c r x", x=130)
        with tc.tile_pool(name="s3om", bufs=2) as s3om, \
             tc.tile_pool(name="s3po", bufs=2, space="PSUM") as s3po, \
             tc.tile_pool(name="s3pt", bufs=2, space="PSUM") as s3pt:
            for rb4 in range(RT // 4):
                pom = s3po.tile([27, 512], F32, tag="pom")
                for k in range(K2):
                    ky, kx = k // 3, k % 3
                    rhs = xp_v[:, rb4 * 4 + ky:rb4 * 4 + ky + 4, kx:kx + 128]
                    nc.tensor.matmul(pom[:].rearrange("o (r x) -> o r x", x=128),
                                     wom_sb[:, k * 27:(k + 1) * 27],
                                     rhs,
                                     start=(k == 0), stop=(k == K2 - 1))
                om_sb = s3om.tile([27, 512], F32, tag="om")
                nc.scalar.activation(om_sb[:], pom[:], ACT.Identity,
                                     bias=bom_sb[:], scale=1.0)
                pt = s3pt.tile([128, 108], F32, tag="pomt")
                for j in range(4):
                    nc.tensor.transpose(pt[:, j * 27:(j + 1) * 27],
                                        om_sb[:, j * 128:(j + 1) * 128],
                                        idf[0:27, 0:27])
                nc.scalar.copy(omt[:, rb4 * 108:(rb4 + 1) * 108], pt[:])

        # ---------- S4: offset math ----------
        with tc.tile_pool(name="s4", bufs=1) as s4:
            cnt = [0]

            def t():
                cnt[0] += 1
                return s4.tile([128, NK], F32, tag=f"s4_{cnt[0]}", name=f"s4_{cnt[0]}")

            omt_v = omt[:].rearrange("p (r ch) -> p r ch", ch=27)
            off18 = omt_v[:, :, 0:18].rearrange("p r (ch two) -> p r ch two", two=2)
            dy = off18[:, :, :, 0]
            dx = off18[:, :, :, 1]
            mm = omt_v[:, :, 18:27]

            def v3(ap):  # [128, NK] tile -> [128, RT, K2] view
                return ap[:].rearrange("p (r k) -> p r k", k=K2)

            py = t(); px = t()
            nc.vector.tensor_tensor(v3(py), dy, v3(rk_sb), AL.add)
            nc.vector.tensor_tensor(v3(px), dx, v3(kxx_sb), AL.add)

            def floor_(src):
                ti = s4.tile([128, NK], I32, tag=f"s4i_{cnt[0]}", name=f"s4i_{cnt[0]}")
                nc.vector.tensor_copy(ti[:], src[:])
                tr = t()
                nc.vector.tensor_copy(tr[:], ti[:])
                tcmp = t()
                nc.vector.tensor_tensor(tcmp[:], tr[:], src[:], AL.is_gt)
                out = t()
                nc.vector.tensor_tensor(out[:], tr[:], tcmp[:], AL.subtract)
                return out

            y0 = floor_(py)
            x0 = floor_(px)
            fy = t(); nc.vector.tensor_tensor(fy[:], py[:], y0[:], AL.subtract)
            fx = t(); nc.vector.tensor_tensor(fx[:], px[:], x0[:], AL.subtract)

            yb = t(); nc.vector.tensor_scalar(yb[:], y0[:], 126.0, 0.0, AL.min, AL.max)
            xb = t(); nc.vector.tensor_scalar(xb[:], x0[:], 126.0, 0.0, AL.min, AL.max)

            msk = t()
            nc.scalar.activation(v3(msk), mm, ACT.Sigmoid)

            def slots(v0, vb, f, mask):
                d = t(); nc.vector.tensor_tensor(d[:], v0[:], vb[:], AL.subtract)
                e0 = t(); nc.vector.tensor_scalar(e0[:], d[:], 0.0, None, AL.is_equal)
                em = t(); nc.vector.tensor_scalar(em[:], d[:], -1.0, None, AL.is_equal)
                ep = t(); nc.vector.tensor_scalar(ep[:], d[:], 1.0, None, AL.is_equal)
                cf = t(); nc.vector.tensor_scalar(cf[:], f[:], -1.0, 1.0, AL.mult, AL.add)
                w0 = t(); w1 = t()
                t1 = t(); nc.vector.tensor_tensor(t1[:], e0[:], cf[:], AL.mult)
                t2 = t(); nc.vector.tensor_tensor(t2[:], em[:], f[:], AL.mult)
                nc.vector.tensor_tensor(w0[:], t1[:], t2[:], AL.add)
                t3 = t(); nc.vector.tensor_tensor(t3[:], e0[:], f[:], AL.mult)
                t4 = t(); nc.vector.tensor_tensor(t4[:], ep[:], cf[:], AL.mult)
                nc.vector.tensor_tensor(w1[:], t3[:], t4[:], AL.add)
                if mask is not None:
                    nc.vector.tensor_tensor(w0[:], w0[:], mask[:], AL.mult)
                    nc.vector.tensor_tensor(w1[:], w1[:], mask[:], AL.mult)
                return w0, w1

            wy0, wy1 = slots(y0, yb, fy, msk)
            wx0, wx1 = slots(x0, xb, fx, None)

            nc.vector.tensor_tensor(w00[:], wy0[:], wx0[:], AL.mult)
            nc.vector.tensor_tensor(w01[:], wy0[:], wx1[:], AL.mult)
            nc.vector.tensor_tensor(w10[:], wy1[:], wx0[:], AL.mult)
            nc.vector.tensor_tensor(w11[:], wy1[:], wx1[:], AL.mult)

            # indices: idx0 = clamp(yb - ybase, 0, HL-2)*128 + xb
            ybl = t()
            nc.vector.tensor_scalar(ybl[:], yb[:], ybase_sb[:, 0:1], None, AL.subtract)
            nc.vector.tensor_scalar(ybl[:], ybl[:], float(HL - 2), 0.0, AL.min, AL.max)
            idxf = t()
            nc.vector.tensor_scalar(idxf[:], ybl[:], 128.0, None, AL.mult)
            nc.vector.tensor_tensor(idxf[:], idxf[:], xb[:], AL.add)
            idx0 = s4.tile([128, NK], I16, tag="idx0")
            idx1 = s4.tile([128, NK], I16, tag="idx1")
            nc.vector.tensor_copy(idx0[:], idxf[:])
            nc.vector.tensor_scalar(idxf[:], idxf[:], 128.0, None, AL.add)
            nc.vector.tensor_copy(idx1[:], idxf[:])

            # wrap-reorg: wr[16G+pp, g*8+a] = idx[16a+pp, g]  for all G
            for src, dst in ((idx0, wr0), (idx1, wr1)):
                dst_v = dst[:].rearrange("q (g a) -> q g a", a=8)
                for a in range(8):
                    nc.sync.dma_start(dst_v[0:16, :, a],
                                      src[16 * a:16 * (a + 1), :])
                for g in range(1, 8):
                    nc.sync.dma_start(dst[16 * g:16 * (g + 1), :], dst[0:16, :])

        # ---------- S5..S8: main loop ----------
        _stage = os.environ.get("DCN_STAGE", "full")
        if _stage != "front":
         with tc.tile_pool(name="mg", bufs=2) as mg, \
             tc.tile_pool(name="mv", bufs=2) as mv, \
             tc.tile_pool(name="mvt", bufs=2) as mvt, \
             tc.tile_pool(name="mo", bufs=2) as mo, \
             tc.tile_pool(name="mpv", bufs=3, space="PSUM") as mpv, \
             tc.tile_pool(name="mpo", bufs=2, space="PSUM") as mpo:
            OCH = 8  # output rows per store DMA
            out_sb = None
            for blk in range(NBLK):
                g0 = mg.tile([128, GRP, 256], BF16, tag="g0")
                g1 = mg.tile([128, GRP, 256], BF16, tag="g1")
                ni = GRP * 128
                s = blk * GRP * 8
                if _stage == "nogather":
                    nc.vector.memset(g0[:], 0.25)
                    nc.vector.memset(g1[:], 0.25)
                else:
                    nc.gpsimd.dma_gather(g0[:], xpd_d.ap(), wr0[:, s:s + GRP * 8],
                                         num_idxs=ni, num_idxs_reg=ni, elem_size=256,
                                         single_packet=False)
                    nc.gpsimd.dma_gather(g1[:], xpd_d.ap(), wr1[:, s:s + GRP * 8],
                                         num_idxs=ni, num_idxs_reg=ni, elem_size=256,
                                         single_packet=False)

                # combine: V = w00*g0A + w01*g0B + w10*g1A + w11*g1B
                # coefs pre-expanded 8-wide so every operand's innermost AP dim
                # is step-1 (unlocks DVE 2x bf16 mode; stride-0 goes to a mid dim)
                V = mv.tile([128, GRP, 128], BF16, tag="V")
                tmp = mv.tile([128, GRP, 128], BF16, tag="Vtmp")
                ce = [mv.tile([128, GRP, 8], BF16, tag=f"ce{i}", name=f"ce{i}")
                      for i in range(4)]
                for i, wt in enumerate((w00, w01, w10, w11)):
                    nc.vector.tensor_copy(
                        ce[i][:], wt[:, blk * GRP:(blk + 1) * GRP].unsqueeze(-1)
                        .broadcast_to((128, GRP, 8)))

                def coefx(i):
                    return ce[i][:].unsqueeze(2).broadcast_to((128, GRP, 16, 8))

                def plane(g, sl):
                    v = g[:].rearrange("p g (s ch cl) -> p g s ch cl", s=2, cl=8)
                    return v[:, :, sl, :, :]

                def v4(ap):
                    return ap.rearrange("p g (ch cl) -> p g ch cl", cl=8)

                nc.vector.tensor_tensor(v4(V[:]), plane(g0, 0), coefx(0), AL.mult)
                nc.vector.tensor_tensor(v4(tmp[:]), plane(g0, 1), coefx(1), AL.mult)
                nc.vector.tensor_tensor(V[:], V[:], tmp[:], AL.add)
                nc.vector.tensor_tensor(v4(tmp[:]), plane(g1, 0), coefx(2), AL.mult)
                nc.vector.tensor_tensor(V[:], V[:], tmp[:], AL.add)
                nc.vector.tensor_tensor(v4(tmp[:]), plane(g1, 1), coefx(3), AL.mult)
                nc.vector.tensor_tensor(V[:], V[:], tmp[:], AL.add)

                # transpose V -> VT [c, (rr,k)*128]
                vt = mvt.tile([128, GRP * 128], BF16, tag="VT")
                for h4 in range((GRP + 3) // 4):
                    pvt = mpv.tile([128, 512], BF16, tag="pvt")
                    n4 = min(4, GRP - h4 * 4)
                    for j in range(n4):
                        g = h4 * 4 + j
                        nc.tensor.transpose(pvt[:, j * 128:(j + 1) * 128],
                                            V[:, g, :], idb[:])
                    nc.scalar.copy(vt[:, h4 * 512:h4 * 512 + n4 * 128],
                                   pvt[:, 0:n4 * 128])

                # main matmul + epilogue
                if blk % (OCH // RB) == 0:
                    out_sb = mo.tile([128, OCH * W], F32, tag="osb")
                for rr in range(RB):
                    po = mpo.tile([128, 128], F32, tag="po")
                    for k in range(K2):
                        g = rr * K2 + k
                        nc.tensor.matmul(po[:], w_sb[:, k * CO:(k + 1) * CO],
                                         vt[:, g * 128:(g + 1) * 128],
                                         start=(k == 0), stop=(k == K2 - 1))
                    ro = (blk * RB + rr) % OCH
                    nc.scalar.activation(out_sb[:, ro * W:(ro + 1) * W], po[:],
                                         ACT.Relu, bias=bv_sb[:], scale=av_sb[:])
                if (blk * RB + RB) % OCH == 0:
                    r0 = (blk * RB + RB) - OCH
                    nc.sync.dma_start(yl_d[:, r0 * W:(r0 + OCH) * W], out_sb[:])

    nc.compile()
    return nc


def _prep_inputs(x, w_om, b_om, w, b, gamma, beta, bn_mean, bn_var):
    """Build the 8 per-core input maps."""
    x = np.ascontiguousarray(x, dtype=np.float32)
    A = (gamma / np.sqrt(bn_var + EPS)).astype(np.float32)
    Bv = ((b - bn_mean) * A + beta).astype(np.float32)
    wom_l = np.ascontiguousarray(
        w_om.reshape(27, C, K2).transpose(1, 2, 0)).astype(np.float32).reshape(C, K2 * 27)
    wl = np.ascontiguousarray(
        w.reshape(CO, C, K2).transpose(1, 2, 0)).astype(ml_dtypes.bfloat16).reshape(C, K2 * CO)
    r = np.arange(RT, dtype=np.float32)[:, None]
    kyv = (np.arange(K2, dtype=np.float32) // 3)[None, :]
    kxv = (np.arange(K2, dtype=np.float32) % 3)[None, :]
    p = np.arange(128, dtype=np.float32)[:, None, None]
    kxx = (np.broadcast_to((kxv - 1)[None], (128, RT, K2))
           + np.broadcast_to(p, (128, RT, K2))).reshape(128, NK).astype(np.float32)
    in_maps = []
    for core in range(8):
        bidx, h = core // 2, core % 2
        ylo = 0 if h == 0 else H - HL
        xp = np.zeros((C, 66, 130), np.float32)
        r0 = 64 * h - 1
        rlo, rhi = max(r0, 0), min(r0 + 66, H)
        xp[:, rlo - r0:rhi - r0, 1:129] = x[bidx, :, rlo:rhi, :]
        xh = np.ascontiguousarray(x[bidx, :, ylo:ylo + HL, :])
        rk = np.broadcast_to((64 * h + r + kyv - 1)[None],
                             (128, RT, K2)).reshape(128, NK)
        in_maps.append(dict(
            xp=np.ascontiguousarray(xp.reshape(C, 66 * 130)),
            xh=xh.reshape(C, HL * W),
            wom=wom_l, bom=b_om.reshape(27, 1).astype(np.float32),
            wl=wl, av=A.reshape(CO, 1), bv=Bv.reshape(CO, 1),
            rk=np.ascontiguousarray(rk, dtype=np.float32),
            kxx=kxx,
            ybase=np.full((128, 1), ylo, np.float32),
        ))
    return in_maps


def kernel(x, w_om, b_om, w, b, gamma, beta, bn_mean, bn_var):
    from concourse.bass_utils import run_bass_kernel_spmd
    if "nc" not in _CACHE:
        _CACHE["nc"] = _build_nc()
    nc = _CACHE["nc"]
    in_maps = _prep_inputs(x, w_om, b_om, w, b, gamma, beta, bn_mean, bn_var)
    res = run_bass_kernel_spmd(nc, in_maps, core_ids=list(range(8)),
                               trace=bool(int(os.environ.get("DCN_TRACE", "0"))))
    out = np.zeros((B, CO, H, W), np.float32)
    for core in range(8):
        bidx, h = core // 2, core % 2
        out[bidx, :, 64 * h:64 * h + 64, :] = \
            res.results[core]["yl"].reshape(CO, RT, W)
    _CACHE["last_result"] = res
    return out

